# revision 103
# baseline (speedup 1.0000x reference)
"""YOLO loss kernel for Trainium2 (Bass/Tile), data-parallel over 8 NeuronCores.

Math (per sample n, cell s; S=14, SS=196, B=2, C=20, D=30):
  t4 = target conf channel (binary 0/1), obj = t4, noobj = 1 - t4
  Per box b: overlap per dim via the interval identity (in S-units)
      S*ox = min(S*tw, S*pw, S*(tw+pw)/2 - |pc - tc|)
  inter = relu(ox)*relu(oy), union = tarea + parea - inter,
  iou_b = inter / (union + 1e-30)   # union==0 implies inter==0 -> iou 0
  sel = iou1 > iou0, selm = sel*t4, s0m = (sel < t4)
  coord = 5 * sum_k s0m*(p_k-t_k)^2 + selm*(p_{5+k}-t_{5+k})^2
  conf  = s0m*(p4-iou0)^2 + selm*(p9-iou1)^2
  noobj = 0.5*(1-t4)*(p4^2+p9^2)
  class = t4 * sum_c (p_c-t_c)^2
  loss = sum(coord+conf+noobj+class)/N

Inputs are cast to bf16 on the host (tolerance 2e-2 >> bf16 error ~1e-4):
halves HBM traffic (DMA roofline ~33us/core) and enables the DVE 2x
(tensor_tensor) / 4x (tensor_scalar) bf16 perf modes. Weights fold into
Act Square scales or binary-mask tricks ((sqrt(w)*mask)^2 = w*mask);
every reduction rides Act Square+accum_out into a [128, NSLOT] fp32
partial the host sums. The 0/0 guard rides the Act fp32 convert as bias.

Layout per core: 512 samples -> 128 partitions x 4 groups. Box channels
arrive as strided DMAs (centers {0,1,5,6}, widths {2,3,7,8}, conf {4,9})
so DVE starts at ~4us. The box/iou/mask phase runs twice over G-halves
(2 groups each) so the first half's conf/coord squares reach the Act
engine ~10us earlier, filling its idle window; the class stream stays
full-width. Pool gets only early-ready, late-consumed subtractions.
"""

import math

import numpy as np

import concourse.mybir as mybir
from concourse import bacc
from concourse.bass_utils import run_bass_kernel_spmd
from concourse.tile import TileContext

F32 = mybir.dt.float32
BF16 = mybir.dt.bfloat16
OP = mybir.AluOpType
AF = mybir.ActivationFunctionType

N, D, S = 4096, 30, 14
SS = S * S          # 196
NCORE = 8
NPC = N // NCORE    # 512 samples per core
P = 128
G = NPC // P        # 4 groups (samples per partition)
H = G // 2          # groups per box-phase block
NSLOT = 12          # [conf+noobj, coord0, coord1] x2 blocks, class x6

SQ5 = math.sqrt(5.0)
SQH = math.sqrt(0.5)

_CACHE = {}


def _build():
    nc = bacc.Bacc("TRN2", target_bir_lowering=False, debug=False)
    pred = nc.dram_tensor("pred", [NPC, D * SS], BF16, kind="ExternalInput")
    tgt = nc.dram_tensor("target", [NPC, D * SS], BF16, kind="ExternalInput")
    out = nc.dram_tensor("out", [P, NSLOT], F32, kind="ExternalOutput")

    pred_r = pred[:, :].rearrange("(g p) d -> p g d", g=G, p=P)
    tgt_r = tgt[:, :].rearrange("(g p) d -> p g d", g=G, p=P)
    # [P, G, box, 5*SS] strided view of the 10 box channels
    pred_b = pred_r[:, :, 0 : 10 * SS].rearrange(
        "p g (b q) -> p g b q", b=2, q=5 * SS)

    with TileContext(nc) as tc:
        with (
            tc.tile_pool(name="big", bufs=1) as big,
            tc.tile_pool(name="cls", bufs=1) as clsp,
            tc.tile_pool(name="tmp", bufs=1) as tmp,
            tc.tile_pool(name="accp", bufs=1) as accp,
        ):
            acc = accp.tile([P, NSLOT], F32)

            def slot(i):
                return acc[:, i : i + 1]

            # ---- DMAs, in arrival-priority order ----
            pbWf = big.tile([P, G, 2, 2 * SS], BF16, tag="pbW", name="pbW")
            pbCf = big.tile([P, G, 2, 2 * SS], BF16, tag="pbC", name="pbC")
            pbF = big.tile([P, G, 2, SS], BF16, tag="pbF", name="pbF")
            tbA = big.tile([P, G, 5, SS], BF16, tag="tbA", name="tbA")
            tbB = big.tile([P, G, 4, SS], BF16, tag="tbB", name="tbB")
            for b in range(2):
                nc.sync.dma_start(out=pbWf[:, :, b, :],
                                  in_=pred_b[:, :, b, 2 * SS : 4 * SS])
            nc.sync.dma_start(
                out=tbA, in_=tgt_r[:, :, 0 : 5 * SS].rearrange(
                    "p g (c s) -> p g c s", c=5, s=SS))
            for b in range(2):
                nc.sync.dma_start(out=pbCf[:, :, b, :],
                                  in_=pred_b[:, :, b, 0 : 2 * SS])
            pbW = pbWf[:, :, :, :].rearrange("p g b (k s) -> p g b k s",
                                             k=2, s=SS)
            pbC = pbCf[:, :, :, :].rearrange("p g b (k s) -> p g b k s",
                                             k=2, s=SS)
            nc.sync.dma_start(
                out=tbB, in_=tgt_r[:, :, 5 * SS : 9 * SS].rearrange(
                    "p g (c s) -> p g c s", c=4, s=SS))
            cls_tiles = {}
            for j in (3, 4, 0, 1, 2):   # j=3 first: earliest Act square
                lo = (10 + 4 * j) * SS
                hi = lo + 4 * SS
                pc = clsp.tile([P, G, 4, SS], BF16, tag=f"pc{j}", name=f"pc{j}")
                tcl = clsp.tile([P, G, 4, SS], BF16, tag=f"tc{j}", name=f"tc{j}")
                nc.sync.dma_start(
                    out=pc, in_=pred_r[:, :, lo:hi].rearrange(
                        "p g (c s) -> p g c s", c=4, s=SS))
                nc.sync.dma_start(
                    out=tcl, in_=tgt_r[:, :, lo:hi].rearrange(
                        "p g (c s) -> p g c s", c=4, s=SS))
                cls_tiles[j] = (pc, tcl)
                if j == 3:   # conf channels: needed later than chunk 3
                    for b in range(2):
                        nc.sync.dma_start(
                            out=pbF[:, :, b, :],
                            in_=pred_b[:, :, b, 4 * SS : 5 * SS])

            t4 = tbA[:, :, 4, :]

            def T(shape, tag, dt=BF16):
                return tmp.tile(shape, dt, tag=tag, name=tag)

            # ================= box phase, per G-half block =================
            C4h = [P, H, 2, 2, SS]
            C2h = [P, H, 2, SS]
            blk_masks = {}
            blk_e8 = {}

            def box_phase(blk):
                g = slice(H * blk, H * blk + H)
                pbWb, pbCb = pbW[:, g], pbC[:, g]
                pbFb = pbF[:, g]
                tCb = tbA[:, g, 0:2, :]
                tWb = tbA[:, g, 2:4, :]
                t4b = tbA[:, g, 4, :]

                def bc2(x2):
                    return x2.unsqueeze(2).broadcast_to((P, H, 2, 2, SS))

                k = str(blk)
                parea2 = T(C2h, "parea2" + k)
                tarea = T([P, H, SS], "tarea" + k)
                s12 = T(C2h, "s12" + k)
                e8 = T([P, H, 8, SS], "e8" + k)
                wno = T([P, H, SS], "wno" + k)
                cn = T([P, H, 4, SS], "cn" + k)
                blk_e8[blk] = e8

                # DVE setup (Pool's s12 reads these)
                puS = T(C4h, "puS" + k)
                pwS = T(C4h, "pwS" + k)
                nc.vector.tensor_scalar(puS, pbWb, S / 2.0, None, OP.mult)
                nc.vector.tensor_scalar(pwS, pbWb, float(S), None, OP.mult)
                nc.vector.tensor_mul(parea2, pbWb[:, :, :, 0, :],
                                     pbWb[:, :, :, 1, :])
                tuS = T(C2h, "tuS" + k)
                twS = T(C2h, "twS" + k)
                nc.vector.tensor_scalar(tuS, tWb, S / 2.0, None, OP.mult)
                nc.vector.tensor_scalar(twS, tWb, float(S), None, OP.mult)
                nc.vector.tensor_scalar(wno, t4b, -SQH, SQH, OP.mult, OP.add)

                # Pool: early-ready, late-consumed pieces
                nc.gpsimd.tensor_tensor(tarea, tbA[:, g, 2, :],
                                        tbA[:, g, 3, :], OP.mult)
                nc.gpsimd.tensor_tensor(
                    s12, parea2,
                    tarea.unsqueeze(2).broadcast_to((P, H, 2, SS)), OP.add)
                nc.gpsimd.tensor_tensor(e8[:, :, 0:2, :], pbCb[:, :, 0, :, :],
                                        tbA[:, g, 0:2, :], OP.subtract)
                nc.gpsimd.tensor_tensor(e8[:, :, 2:4, :], pbWb[:, :, 0, :, :],
                                        tbA[:, g, 2:4, :], OP.subtract)
                nc.gpsimd.tensor_tensor(e8[:, :, 4:6, :], pbCb[:, :, 1, :, :],
                                        tbB[:, g, 0:2, :], OP.subtract)
                nc.gpsimd.tensor_tensor(e8[:, :, 6:8, :], pbWb[:, :, 1, :, :],
                                        tbB[:, g, 2:4, :], OP.subtract)
                nc.gpsimd.tensor_tensor(
                    cn[:, :, 2:4, :], pbFb,
                    wno.unsqueeze(2).broadcast_to((P, H, 2, SS)), OP.mult)

                # DVE: overlap chain
                dC = T(C4h, "x4a" + k)
                nc.vector.tensor_tensor(dC, pbCb, bc2(tCb), OP.subtract)
                adC = T(C4h, "x4b" + k)
                nc.scalar.activation(adC, dC, AF.Abs)          # Act
                h0 = T(C4h, "x4c" + k)
                nc.vector.tensor_tensor(h0, puS, bc2(tuS), OP.add)
                m4 = T(C4h, "x4d" + k)
                nc.vector.tensor_tensor(m4, pwS, bc2(twS), OP.min)
                h1 = T(C4h, "x4a" + k)     # reuses dC
                nc.vector.tensor_sub(h1, h0, adC)
                o4 = T(C4h, "x4b" + k)     # reuses adC
                nc.vector.tensor_tensor(o4, m4, h1, OP.min)
                orr = T(C4h, "x4c" + k)    # reuses h0
                if blk == 0:
                    nc.scalar.activation(orr, o4, AF.Relu, scale=1.0 / S)
                else:
                    nc.vector.tensor_scalar(orr, o4, 0.0, 1.0 / S,
                                            OP.max, OP.mult)

                with tc.high_priority():
                    inter2 = T(C2h, "inter2" + k)
                    nc.vector.tensor_mul(inter2, orr[:, :, :, 0, :],
                                         orr[:, :, :, 1, :])
                    union2 = T(C2h, "c2a" + k)
                    nc.vector.tensor_sub(union2, s12, inter2)
                    uf32 = T([P, H, 2 * SS], "uf32" + k, dt=F32)
                    nc.scalar.activation(       # Act: cvt + 0/0 guard bias
                        uf32.rearrange("p g (c s) -> p g c s", c=2, s=SS),
                        union2, AF.Copy, bias=1e-30)
                    rf32 = T([P, H, 2 * SS], "rf32" + k, dt=F32)
                    nc.vector.reciprocal_approx_fast(out=rf32, in_=uf32)
                    iou2 = T(C2h, "iou2" + k)
                    nc.vector.tensor_mul(
                        iou2, inter2,
                        rf32.rearrange("p g (c s) -> p g c s", c=2, s=SS))

                    sel = T([P, H, SS], "sel" + k)
                    nc.vector.tensor_tensor(
                        sel, iou2[:, :, 1, :], iou2[:, :, 0, :], OP.is_gt)
                    masks2 = T(C2h, "masks2" + k)
                    nc.vector.tensor_tensor(
                        masks2[:, :, 0, :], sel, t4b, OP.is_lt)
                    nc.vector.tensor_mul(masks2[:, :, 1, :], sel, t4b)
                    blk_masks[blk] = masks2

                    f2 = T(C2h, "f2" + k)
                    nc.vector.tensor_sub(f2, pbFb, iou2)
                    nc.vector.tensor_mul(cn[:, :, 0:2, :], f2, masks2)
                nc.scalar.activation(cn, cn, AF.Square, accum_out=slot(3 * blk))

            def coord_finish(blk, h, ttr=False, eng=None):
                e8 = blk_e8[blk]
                masks2 = blk_masks[blk]
                part = e8[:, :, 4 * h : 4 * h + 4, :]
                if not ttr:
                    (eng or nc.vector).tensor_tensor(
                        part, part,
                        masks2[:, :, h, :].unsqueeze(2).broadcast_to(
                            (P, H, 4, SS)), OP.mult)
                    nc.scalar.activation(part, part, AF.Square, scale=SQ5,
                                         accum_out=slot(3 * blk + 1 + h))
                else:
                    import bass_rust as _br2
                    me = T([P, H, 4, SS], f"mec{blk}{h}")
                    nc.vector.tensor_mul(
                        me, part,
                        masks2[:, :, h, :].unsqueeze(2).broadcast_to(
                            (P, H, 4, SS)))
                    nc.vector.tensor_scalar(me, me, SQ5, None, OP.mult)
                    nc.vector.tensor_mul(me, me, part)
                    nc.vector.tensor_scalar(me, me, SQ5, None, OP.mult)
                    nc.vector.reduce_sum(slot(3 * blk + 1 + h), me,
                                         _br2.AxisListType.XYZ)

            # ================= class stream (full width) =================
            t4b4 = t4.unsqueeze(2).broadcast_to((P, G, 4, SS))
            cls_slot = {0: 6, 1: 7, 3: 8, 4: 9}

            def cls_finish(j):
                pc, _ = cls_tiles[j]
                nc.vector.tensor_tensor(pc, pc, t4b4, OP.mult)
                nc.scalar.activation(pc, pc, AF.Square,
                                     accum_out=slot(cls_slot[j]))

            def cls_sub(j):
                pc, tcl = cls_tiles[j]
                nc.vector.tensor_sub(pc, pc, tcl)

            # ---- emission: block 0 chain, class flow, block 1 chain ----
            box_phase(0)
            cls_sub(3)
            cls_finish(3)
            box_phase(1)
            coord_finish(0, 0)
            coord_finish(0, 1)
            cls_sub(0)
            cls_finish(0)
            cls_sub(4)
            cls_finish(4)
            pc1x, tc1x = cls_tiles[1]
            nc.gpsimd.tensor_tensor(pc1x, pc1x, tc1x, OP.subtract)  # Pool
            coord_finish(1, 0)
            coord_finish(1, 1)
            # last chunk in 2-ch halves: short final squares
            pc2, tc2 = cls_tiles[2]
            t4b2 = t4.unsqueeze(2).broadcast_to((P, G, 2, SS))
            import bass_rust as _br
            for half in range(2):
                hs = slice(2 * half, 2 * half + 2)
                e2h = pc2[:, :, hs, :]
                nc.vector.tensor_tensor(e2h, e2h, tc2[:, :, hs, :],
                                        OP.subtract)
                if half == 0:
                    nc.vector.tensor_tensor(e2h, e2h, t4b2, OP.mult)
                    nc.scalar.activation(e2h, e2h, AF.Square,
                                         accum_out=slot(10))
                else:
                    # Act is backlogged by now and DVE idle: square+reduce
                    # the very last piece on DVE instead
                    me2 = T([P, G, 2, SS], f"me2_{half}")
                    nc.vector.tensor_tensor(me2, e2h, t4b2, OP.mult)
                    nc.vector.tensor_tensor(me2, me2, e2h, OP.mult)
                    nc.vector.reduce_sum(slot(10 + half), me2,
                                         _br.AxisListType.XYZ)

            cls_finish(1)      # me waits Pool's late sub; emit last
            nc.sync.dma_start(out=out[:, :], in_=acc)
    nc.compile()
    return nc


def _get_nc():
    if "nc" not in _CACHE:
        _CACHE["nc"] = _build()
    return _CACHE["nc"]


def kernel(pred: np.ndarray, target: np.ndarray) -> np.ndarray:
    import ml_dtypes
    bf16 = ml_dtypes.bfloat16
    nc = _get_nc()
    p16 = np.ascontiguousarray(pred).reshape(N, D * SS).astype(bf16)
    t16 = np.ascontiguousarray(target).reshape(N, D * SS).astype(bf16)
    in_maps = []
    for k in range(NCORE):
        sl = slice(k * NPC, (k + 1) * NPC)
        in_maps.append({"pred": p16[sl], "target": t16[sl]})
    res = run_bass_kernel_spmd(nc, in_maps, core_ids=list(range(NCORE)))
    total = sum(float(r["out"].astype(np.float64).sum()) for r in res.results)
    return np.float32(total / N)
